# revision 28
# baseline (speedup 1.0000x reference)
"""Trainium2 Bass kernel for nn_MultiHeadAttention (B=4, S=2048, D=1024, H=16, causal, RoPE).

Sharding: 8 cores = 4 batches x 2 head-groups (8 heads each).
Each core computes q/k/v projections for its 512 head-dims, causal attention
for its 8 heads over its batch, and a partial o_proj; the host sums the two
partial o_proj outputs per batch (tensor-parallel reduce done host-side).

v3 layout: all inputs are pre-cast to bf16 on the host. q/k tiles keep the
natural interleaved head-dim order, two heads per 128-partition tile, so each
head's score matmul is a single K=64 matmul (two heads run concurrently in
the PE array via row tiling). RoPE becomes: ACT evacuates the projection
PSUM to bf16, one DVE stream_shuffle swaps even/odd partition pairs, then
two bf16 multiplies (cos / signed-sin tables) and an add - all at DVE 2x rate.
Scores for both heads of a tile go to one 2-bank PSUM tile and are exp'd by
a single ACT instruction ([128, 2, N]), halving ACT instruction count.
Softmax denominator falls out of the PV matmul via a ones column in v.
"""

import contextlib
import ctypes
import sys
import types

sys.path.insert(0, "/opt/trn_rl_repo")

import numpy as np

import concourse.bass as bass
import concourse.tile as tile
from concourse import bass_utils, mybir
from concourse.vector_clock import ScopedClock

B, S, D = 4, 2048, 1024
H = 16
DK = 64
HG = 2              # head groups (cores per batch)
HL = H // HG        # heads per core = 8
DH = HL * DK        # head dims per core = 512
THETA = 10000.0
N_CORES = 8

F32 = mybir.dt.float32
BF16 = mybir.dt.bfloat16

_PATCHED = False
_NC_CACHE = {}


def _install_patches():
    """Environment fixes: split drain waits (this walrus rejects >2 waits per
    instruction), skip remote artifact upload, install the NTFF profile hook."""
    global _PATCHED
    if _PATCHED:
        return
    _PATCHED = True

    def patched_drain_and_barrier(self, tick_clock, wait_clock):
        nc = self.nc
        scratch = mybir.InstDrain(name="drain-wait-scratch", ins=[], outs=[])
        scratch.sync_info = mybir.SyncInfo(on_wait=[], on_update=[])
        scratch.engine = mybir.EngineType.SP
        wait_clock.add_sem_waits(scratch, ScopedClock({None: tick_clock.global_clock}))
        by_name = {s.name: s for s in self.sems.allocated().values()}
        for ent in scratch.sync_info.on_wait:
            nc.sync.wait_ge(by_name[ent.ant_name], ent.wait_value)
        nc.sync.drain()
        nc.all_engine_barrier()
        popped = nc._tile_sem_poison_stack.pop()
        assert popped is self._sem_poison
        nc.clear_and_free_semaphores(list(self.sems.allocated().values()))
        nc.all_engine_barrier()

    tile.TileContext._drain_and_barrier = patched_drain_and_barrier

    # this walrus accepts at most ONE sync wait per instruction: hoist excess
    # waits onto same-engine InstNoOp carriers just before the instruction.
    # Safe because Tile waits only ever point backward in the schedule order.
    orig_cal = tile.TileContext._commit_and_lower
    ws_counter = [0]

    def patched_commit_and_lower(self, inst, original_block, old_bb_map,
                                 bb_to_exit_bb):
        si = getattr(inst, "sync_info", None)
        if si is not None and si.on_wait and len(si.on_wait) > 1:
            waits = list(si.on_wait)
            for w in waits[:-1]:
                ws_counter[0] += 1
                nop = mybir.InstNoOp(
                    name=f"waitsplit-{ws_counter[0]}",
                    sync_info=mybir.SyncInfo(on_wait=[w], on_update=[]),
                    bass_nofuse=True,
                    engine=inst.engine,
                )
                self._commit_instruction(nop, lazy_reg_writes=False)
            inst.sync_info = mybir.SyncInfo(
                on_wait=[waits[-1]], on_update=list(si.on_update))
        return orig_cal(self, inst, original_block, old_bb_map, bb_to_exit_bb)

    tile.TileContext._commit_and_lower = patched_commit_and_lower
    bass_utils.upload_artifacts = lambda tmpdir: str(tmpdir)

    so_path = "/opt/axon/libaxon_pjrt.so"
    hook = None
    try:
        lib = ctypes.CDLL(so_path)
        if hasattr(lib, "axon_start_nrt_profile"):
            lib.axon_start_nrt_profile.argtypes = [
                ctypes.POINTER(ctypes.c_int64), ctypes.c_size_t]
            lib.axon_start_nrt_profile.restype = ctypes.c_int64
            lib.axon_stop_nrt_profile.argtypes = [ctypes.c_char_p]
            lib.axon_stop_nrt_profile.restype = ctypes.c_int64

            @contextlib.contextmanager
            def _hook(output_dir, device_ids):
                import jax
                jax.devices()
                if device_ids:
                    ids = (ctypes.c_int64 * len(device_ids))(*device_ids)
                    rc = lib.axon_start_nrt_profile(ids, len(device_ids))
                else:
                    rc = lib.axon_start_nrt_profile(None, 0)
                if rc != 0:
                    raise RuntimeError(f"axon_start_nrt_profile rc={rc}")
                try:
                    yield
                finally:
                    n = lib.axon_stop_nrt_profile(str(output_dir).encode())
                    print(f"ntff profile: {n} file(s) -> {output_dir}")

            hook = _hook
    except OSError:
        pass

    import antenv
    mod = types.ModuleType("antenv.axon_hooks")
    mod.get_axon_ntff_profile_hook = lambda: hook
    mod.set_axon_ntff_profile_hook = lambda h: None
    sys.modules["antenv.axon_hooks"] = mod
    antenv.axon_hooks = mod


SWAP_MASK = [i ^ 1 for i in range(32)]   # pair swap 2i <-> 2i+1 per quadrant


def build_nc(seq=S):
    """One SPMD program; per-core differences are input data only."""
    QC = 512                      # query-chunk width (= one PSUM bank of f32)
    n_qc = seq // QC              # query chunks
    n_kt = seq // 128             # key tiles
    n_st = seq // 128             # s tiles (o_proj M)
    KT = 8                        # contraction tiles for projections (D/128)

    nc = bass.Bass(target_bir_lowering=False)

    xT_d = nc.dram_tensor("xT", [D, seq], BF16, kind="ExternalInput")
    wq_d = nc.dram_tensor("wq", [D, DH], BF16, kind="ExternalInput")
    wk_d = nc.dram_tensor("wk", [D, DH], BF16, kind="ExternalInput")
    wv_d = nc.dram_tensor("wv", [D, DH], BF16, kind="ExternalInput")
    wo_d = nc.dram_tensor("wo", [DH, D], BF16, kind="ExternalInput")
    cosb_d = nc.dram_tensor("cosb", [128, seq], BF16, kind="ExternalInput")
    sinb_d = nc.dram_tensor("sinb", [128, seq], BF16, kind="ExternalInput")
    wm_d = nc.dram_tensor("wmask", [128, 128], BF16, kind="ExternalInput")
    out_d = nc.dram_tensor("out", [seq, D], BF16, kind="ExternalOutput")

    with tile.TileContext(nc) as tc:
        with contextlib.ExitStack() as ctx:
            res = ctx.enter_context(tc.tile_pool(name="res", bufs=1))
            ropet = ctx.enter_context(tc.tile_pool(name="ropet", bufs=3))
            pts = ctx.enter_context(tc.tile_pool(name="pts", bufs=3))
            nrm = ctx.enter_context(tc.tile_pool(name="nrm", bufs=2))
            psum = ctx.enter_context(
                tc.tile_pool(name="psum", bufs=4, space="PSUM"))

            # ---- load (host pre-casts everything to bf16) -------------------
            # DMA issue costs ~630ns per dma_start on the issuing engine, so
            # spread issue across engine queues and split xT so the first
            # projection only waits for the first half of x.
            def load_bf16(dram, cols, n_tiles, name, eng, chunks=1):
                tiles = []
                for k in range(n_tiles):
                    bt = res.tile([128, cols], BF16, name=f"{name}{k}",
                                  tag=f"{name}{k}")
                    tiles.append(bt)
                for ch in range(chunks):
                    cw = cols // chunks
                    for k in range(n_tiles):
                        eng.dma_start(
                            tiles[k][:, ch * cw:(ch + 1) * cw],
                            dram[k * 128:(k + 1) * 128, ch * cw:(ch + 1) * cw])
                return tiles

            wq = load_bf16(wq_d, DH, 8, "wq", nc.scalar)
            xT = load_bf16(xT_d, seq, 8, "xT", nc.sync, chunks=2)
            cosb = res.tile([128, seq], BF16, name="cosb", tag="cosb")
            nc.gpsimd.dma_start(cosb[:], cosb_d[:])
            sinb = res.tile([128, seq], BF16, name="sinb", tag="sinb")
            nc.gpsimd.dma_start(sinb[:], sinb_d[:])
            wm = res.tile([128, 128], BF16, name="wm", tag="wm")
            nc.gpsimd.dma_start(wm[:], wm_d[:])
            wk = load_bf16(wk_d, DH, 8, "wk", nc.sync)
            wv = load_bf16(wv_d, DH, 8, "wv", nc.sync)
            wo = load_bf16(wo_d, D, 4, "wo", nc.gpsimd)

            # ---- persistent result tiles -----------------------------------
            # qT/kT tile t: rows 0-63 = head 2t dims, rows 64-127 = head 2t+1
            # (natural interleaved even/odd order within each head).
            qT = [res.tile([128, seq], BF16, name=f"qT{m}", tag=f"qT{m}")
                  for m in range(4)]
            kTt = [res.tile([128, seq], BF16, name=f"kT{m}", tag=f"kT{m}")
                   for m in range(4)]
            v_sb = [res.tile([128, HL, DK + 1], BF16, name=f"v{t}",
                             tag=f"v{t}") for t in range(n_st)]
            aoT = [res.tile([128, seq], BF16, name=f"aoT{t}", tag=f"aoT{t}")
                   for t in range(4)]
            SCALE = 1.0 / np.sqrt(np.float32(DK))

            rs = res.tile([32, QC], F32, name="rs", tag="rs")
            nc.vector.memset(rs[:], 0.0)
            rs2 = res.tile([1, QC], F32, name="rs2", tag="rs2")

            # ---- emission units --------------------------------------------
            def proj_qk_unit(w_tiles, dst, t, qc, evac_on_act=True):
                # 8 MMs for one 128-row tile (2 heads), then rope:
                # psum->bf16 evacuate (ACT when ACT is idle, DVE during the
                # exp-bound attention phases), DVE pair-swap shuffle
                # + 2 mul + add at bf16 2x rate.
                pst = psum.tile([128, QC], F32, tag="ps", name="pj")
                for k in range(KT):
                    nc.tensor.matmul(
                        pst[:],
                        w_tiles[k][:, t * 128:(t + 1) * 128],
                        xT[k][:, qc * QC:(qc + 1) * QC],
                        start=(k == 0), stop=(k == KT - 1))
                t0 = ropet.tile([128, QC], BF16, tag="rope0", name="t0")
                if evac_on_act:
                    nc.scalar.copy(t0[:], pst[:])
                else:
                    nc.vector.tensor_copy(t0[:], pst[:])
                t1 = ropet.tile([128, QC], BF16, tag="rope1", name="t1")
                nc.vector.stream_shuffle(t1[:], t0[:], SWAP_MASK)
                cs = cosb[:, qc * QC:(qc + 1) * QC]
                sn = sinb[:, qc * QC:(qc + 1) * QC]
                ta = ropet.tile([128, QC], BF16, tag="rope2", name="ta")
                nc.vector.tensor_mul(ta[:], t0[:], cs)
                tb = ropet.tile([128, QC], BF16, tag="rope3", name="tb")
                nc.vector.tensor_mul(tb[:], t1[:], sn)
                nc.vector.tensor_add(
                    dst[t][:, qc * QC:(qc + 1) * QC], ta[:], tb[:])

            def v_unit(st_i):
                # 8 MMs: v projection for one s-tile + strided copy + ones.
                pst = psum.tile([128, DH], F32, tag="ps", name="pv_proj")
                for k in range(KT):
                    nc.tensor.matmul(
                        pst[:],
                        xT[k][:, st_i * 128:(st_i + 1) * 128],
                        wv[k][:],
                        start=(k == 0), stop=(k == KT - 1))
                vt = v_sb[st_i]
                nc.vector.tensor_copy(
                    vt[:, :, 0:DK],
                    pst[:].rearrange("p (h d) -> p h d", h=HL))
                nc.vector.memset(vt[:, :, DK:DK + 1], 1.0)

            def oproj_unit(st_i, oc):
                # 4 MMs: one o_proj output tile.
                pso = psum.tile([128, 512], F32, tag="ps", name="pso")
                for k4 in range(4):
                    nc.tensor.matmul(
                        pso[:],
                        aoT[k4][:, st_i * 128:(st_i + 1) * 128],
                        wo[k4][:, oc * 512:(oc + 1) * 512],
                        start=(k4 == 0), stop=(k4 == 3))
                ot = pts.tile([128, 512], BF16, tag="ot", name="ot", bufs=3)
                nc.vector.tensor_copy(ot[:], pso[:])
                nc.sync.dma_start(
                    out_d[st_i * 128:(st_i + 1) * 128,
                          oc * 512:(oc + 1) * 512],
                    ot[:])

            # fill queue: (mm_cost, closure). Pumped between attention tiles.
            fills = []
            fill_pos = [0]
            mm_credit = [0.0]

            def pump(n_mms):
                mm_credit[0] += n_mms
                while (fill_pos[0] < len(fills)
                       and mm_credit[0] >= fills[fill_pos[0]][0]):
                    cost, fn = fills[fill_pos[0]]
                    fn()
                    mm_credit[0] -= cost
                    fill_pos[0] += 1

            def flush_fills(upto=None):
                end = len(fills) if upto is None else upto
                while fill_pos[0] < end:
                    fills[fill_pos[0]][1]()
                    fill_pos[0] += 1
                mm_credit[0] = 0.0

            # ---- attention building blocks ---------------------------------
            def act_reciprocal(out, in_):
                # 1/d = exp(-ln d): Ln and Exp share one ACT table set, so no
                # ~2.7us table reload per call; DVE RECIPROCAL measures 3.3us
                # per [1,512] (iterative divide) so ACT is the fast path.
                nc.scalar.activation(
                    rs2[0:1, :], in_, mybir.ActivationFunctionType.Ln)
                nc.scalar.activation(
                    out, rs2[0:1, :], mybir.ActivationFunctionType.Exp,
                    scale=-1.0)

            def emit_norm(pv_pair, t, qc):
                for i in range(2):
                    h = 2 * t + i
                    act_reciprocal(rs[0:1, :], pv_pair[i][DK:DK + 1, :])
                    rbc = nrm.tile([64, QC], F32, tag="rbc", name="rbc")
                    nc.vector.stream_shuffle(rbc[0:32, :], rs[:, :], [0] * 32)
                    nc.vector.stream_shuffle(rbc[32:64, :], rs[:, :], [0] * 32)
                    nc.vector.tensor_mul(
                        aoT[t][i * 64:i * 64 + 64, qc * QC:(qc + 1) * QC],
                        pv_pair[i][0:DK, :], rbc[:])

            def att_block(qc, t, pump_rate=2):
                qt, kt_t = qT[t], kTt[t]
                pv_ps = [psum.tile([DK + 1, QC], F32, tag="ps", name="pv_ps")
                         for _ in range(2)]
                kt_hi = min(n_kt, 4 * (qc + 1))

                def emit_sc(kt):
                    r = kt - 4 * qc
                    c0 = 128 * r if r > 0 else 0
                    sc2 = psum.tile([128, 2, QC], F32, tag="sc2", name="sc2",
                                    bufs=2)
                    for i in range(2):
                        rb = 64 * i
                        nc.tensor.matmul(
                            sc2[:, i, c0:QC],
                            kt_t[rb:rb + 64, kt * 128:(kt + 1) * 128],
                            qt[rb:rb + 64, qc * QC + c0:(qc + 1) * QC],
                            start=True, stop=True, tile_position=(rb, 0))
                    return kt, c0, sc2

                def emit_px(kt, c0, sc2):
                    r = kt - 4 * qc
                    pt2 = pts.tile([128, 2, QC], BF16, tag="pts", name="pt2")
                    nc.scalar.activation(
                        pt2[:, :, c0:QC], sc2[:, :, c0:QC],
                        mybir.ActivationFunctionType.Exp, scale=SCALE)
                    if r >= 0:
                        for i in range(2):
                            nc.vector.tensor_mul(
                                pt2[:, i, c0:c0 + 128], pt2[:, i, c0:c0 + 128],
                                wm[:, :])
                    for i in range(2):
                        h = 2 * t + i
                        nc.tensor.matmul(
                            pv_ps[i][:, c0:QC],
                            v_sb[kt][:, h, :],
                            pt2[:, i, c0:QC],
                            start=(kt == 0), stop=(kt == kt_hi - 1))

                prev = None
                for kt in range(kt_hi):
                    cur = emit_sc(kt)
                    if prev is not None:
                        emit_px(*prev)
                        pump(pump_rate)
                    prev = cur
                emit_px(*prev)
                pump(pump_rate)
                emit_norm(pv_ps, t, qc)

            # ---- schedule ---------------------------------------------------
            # pre-phase: tiles 0-1 q/k projections + first v tiles (dense)
            for qc in range(n_qc):
                proj_qk_unit(wq, qT, 0, qc)
                proj_qk_unit(wq, qT, 1, qc)
            for qc in range(n_qc):
                proj_qk_unit(wk, kTt, 0, qc)
                proj_qk_unit(wk, kTt, 1, qc)
            for t in range(min(4, n_st)):
                v_unit(t)

            # fill queue for phase A: remaining v tiles, tile 2-3 projections
            for t in range(4, n_st):
                fills.append((8, lambda t=t: v_unit(t)))
            v_fill_end = len(fills)
            for qc in range(n_qc):
                fills.append(
                    (8, lambda qc=qc: proj_qk_unit(wq, qT, 2, qc, False)))
                fills.append(
                    (8, lambda qc=qc: proj_qk_unit(wq, qT, 3, qc, False)))
            for qc in range(n_qc):
                fills.append(
                    (8, lambda qc=qc: proj_qk_unit(wk, kTt, 2, qc, False)))
                fills.append(
                    (8, lambda qc=qc: proj_qk_unit(wk, kTt, 3, qc, False)))
            qk1_fill_end = len(fills)

            def ensure_v(qc):
                # v tiles up to 4*(qc+1) must exist before attention reads them
                need = min(4 * (qc + 1), n_st) - 4
                if need > 0:
                    flush_fills(upto=min(need, v_fill_end))

            # phase A: tiles 0-1 attention (+ tiles 2-3 qc0/qc1 at the end)
            for qc in range(n_qc):
                ensure_v(qc)
                att_block(qc, 0, pump_rate=3)
                att_block(qc, 1, pump_rate=3)
            flush_fills(upto=qk1_fill_end)   # tile 2-3 q/k must be ready now
            for qc in range(min(2, n_qc)):
                ensure_v(qc)
                att_block(qc, 2)
                att_block(qc, 3)
                for st_i in range(4 * qc, min(4 * (qc + 1), n_st)):
                    for oc in range(2):
                        fills.append(
                            (4, lambda s=st_i, o=oc: oproj_unit(s, o)))

            # phase B: heavy tile 2-3 chunks with o_proj as PE filler.
            # qc=3 runs before qc=2 so the final qc's o_proj DMAs overlap
            # qc=2's attention instead of trailing the whole kernel.
            for qc in [3, 2][:max(0, n_qc - 2)]:
                ensure_v(qc)
                att_block(qc, 2)
                att_block(qc, 3)
                for st_i in range(4 * qc, min(4 * (qc + 1), n_st)):
                    for oc in range(2):
                        fills.append(
                            (4, lambda s=st_i, o=oc: oproj_unit(s, o)))
            flush_fills()
    return nc


def prepare_inputs(x, q_proj, k_proj, v_proj, o_proj, token_positions, seq=S):
    """Shard + lay out host-side. Returns one in_map per core."""
    from ml_dtypes import bfloat16
    x = np.asarray(x, dtype=np.float32)
    q_proj = np.asarray(q_proj, dtype=np.float32)
    k_proj = np.asarray(k_proj, dtype=np.float32)
    v_proj = np.asarray(v_proj, dtype=np.float32)
    o_proj = np.asarray(o_proj, dtype=np.float32)
    pos = np.asarray(token_positions)

    # rope tables (exactly mirrors reference._rope_tables + gather)
    dims = np.arange(0, DK, 2, dtype=np.float32)
    freqs = 1.0 / THETA ** (dims / DK)
    t = np.arange(2048, dtype=np.float32)
    angles = np.outer(t, freqs)                      # (2048, 32)
    cos_tab = np.cos(angles)[pos].astype(np.float32)  # (seq, 32)
    sin_tab = np.sin(angles)[pos].astype(np.float32)
    # interleaved layout: row r uses frequency (r % 64) // 2; sin sign is
    # -1 on even rows (out_e = e*cos - o*sin) and +1 on odd rows.
    cos_il = np.repeat(np.ascontiguousarray(cos_tab.T), 2, axis=0)  # (64, seq)
    sin_il = np.repeat(np.ascontiguousarray(sin_tab.T), 2, axis=0)
    sign = np.where(np.arange(64) % 2 == 0, -1.0, 1.0)[:, None].astype(
        np.float32)
    sin_il = sin_il * sign
    cosb = np.tile(cos_il, (2, 1))                   # (128, seq)
    sinb = np.tile(sin_il, (2, 1))

    # shifted causal mask for the 128-wide diagonal block: wm[k, c] = c >= k
    kk = np.arange(128)[:, None]
    cc = np.arange(128)[None, :]
    wm = (cc >= kk).astype(np.float32)

    in_maps = []
    for c in range(N_CORES):
        b, hg = c // 2, c % 2
        hslice = slice(hg * DH, (hg + 1) * DH)
        in_maps.append({
            "xT": np.ascontiguousarray(x[b, :seq, :].T).astype(bfloat16),
            "wq": np.ascontiguousarray(q_proj[:, hslice]).astype(bfloat16),
            "wk": np.ascontiguousarray(k_proj[:, hslice]).astype(bfloat16),
            "wv": np.ascontiguousarray(v_proj[:, hslice]).astype(bfloat16),
            "wo": np.ascontiguousarray(o_proj[hslice, :]).astype(bfloat16),
            "cosb": cosb[:, :seq].astype(bfloat16),
            "sinb": sinb[:, :seq].astype(bfloat16),
            "wmask": wm.astype(bfloat16),
        })
    return in_maps


def run(inputs, seq=S, trace=False, tmpdir=None):
    _install_patches()
    if seq not in _NC_CACHE:
        _NC_CACHE[seq] = build_nc(seq)
    nc = _NC_CACHE[seq]
    in_maps = prepare_inputs(**inputs, seq=seq)
    kw = {}
    if trace:
        kw = dict(trace=True, tmpdir=tmpdir)
    res = bass_utils.run_bass_kernel_spmd(
        nc, in_maps, core_ids=list(range(N_CORES)), **kw)
    parts = [np.asarray(res.results[c]["out"], dtype=np.float32)
             for c in range(N_CORES)]
    out = np.stack([parts[2 * b] + parts[2 * b + 1] for b in range(B)])
    return out, res


def kernel(x, q_proj, k_proj, v_proj, o_proj, token_positions):
    out, _ = run(dict(x=x, q_proj=q_proj, k_proj=k_proj, v_proj=v_proj,
                      o_proj=o_proj, token_positions=token_positions))
    return out


# revision 29
# speedup vs baseline: 1.0568x; 1.0568x over previous
"""Trainium2 Bass kernel for nn_MultiHeadAttention (B=4, S=2048, D=1024, H=16, causal, RoPE).

Sharding: 8 cores = 4 batches x 2 head-groups (8 heads each).
Each core computes q/k/v projections for its 512 head-dims, causal attention
for its 8 heads over its batch, and a partial o_proj; the host sums the two
partial o_proj outputs per batch (tensor-parallel reduce done host-side).

v3 layout: all inputs are pre-cast to bf16 on the host. q/k tiles keep the
natural interleaved head-dim order, two heads per 128-partition tile, so each
head's score matmul is a single K=64 matmul (two heads run concurrently in
the PE array via row tiling). RoPE becomes: ACT evacuates the projection
PSUM to bf16, one DVE stream_shuffle swaps even/odd partition pairs, then
two bf16 multiplies (cos / signed-sin tables) and an add - all at DVE 2x rate.
Scores for both heads of a tile go to one 2-bank PSUM tile and are exp'd by
a single ACT instruction ([128, 2, N]), halving ACT instruction count.
Softmax denominator falls out of the PV matmul via a ones column in v.
"""

import contextlib
import ctypes
import sys
import types

sys.path.insert(0, "/opt/trn_rl_repo")

import numpy as np

import concourse.bass as bass
import concourse.tile as tile
from concourse import bass_utils, mybir
from concourse.vector_clock import ScopedClock

B, S, D = 4, 2048, 1024
H = 16
DK = 64
HG = 2              # head groups (cores per batch)
HL = H // HG        # heads per core = 8
DH = HL * DK        # head dims per core = 512
THETA = 10000.0
N_CORES = 8

F32 = mybir.dt.float32
BF16 = mybir.dt.bfloat16

_PATCHED = False
_NC_CACHE = {}


def _install_patches():
    """Environment fixes: split drain waits (this walrus rejects >2 waits per
    instruction), skip remote artifact upload, install the NTFF profile hook."""
    global _PATCHED
    if _PATCHED:
        return
    _PATCHED = True

    def patched_drain_and_barrier(self, tick_clock, wait_clock):
        nc = self.nc
        scratch = mybir.InstDrain(name="drain-wait-scratch", ins=[], outs=[])
        scratch.sync_info = mybir.SyncInfo(on_wait=[], on_update=[])
        scratch.engine = mybir.EngineType.SP
        wait_clock.add_sem_waits(scratch, ScopedClock({None: tick_clock.global_clock}))
        by_name = {s.name: s for s in self.sems.allocated().values()}
        for ent in scratch.sync_info.on_wait:
            nc.sync.wait_ge(by_name[ent.ant_name], ent.wait_value)
        nc.sync.drain()
        nc.all_engine_barrier()
        popped = nc._tile_sem_poison_stack.pop()
        assert popped is self._sem_poison
        nc.clear_and_free_semaphores(list(self.sems.allocated().values()))
        nc.all_engine_barrier()

    tile.TileContext._drain_and_barrier = patched_drain_and_barrier

    # this walrus accepts at most ONE sync wait per instruction: hoist excess
    # waits onto same-engine InstNoOp carriers just before the instruction.
    # Safe because Tile waits only ever point backward in the schedule order.
    orig_cal = tile.TileContext._commit_and_lower
    ws_counter = [0]

    def patched_commit_and_lower(self, inst, original_block, old_bb_map,
                                 bb_to_exit_bb):
        si = getattr(inst, "sync_info", None)
        if si is not None and si.on_wait and len(si.on_wait) > 1:
            waits = list(si.on_wait)
            for w in waits[:-1]:
                ws_counter[0] += 1
                nop = mybir.InstNoOp(
                    name=f"waitsplit-{ws_counter[0]}",
                    sync_info=mybir.SyncInfo(on_wait=[w], on_update=[]),
                    bass_nofuse=True,
                    engine=inst.engine,
                )
                self._commit_instruction(nop, lazy_reg_writes=False)
            inst.sync_info = mybir.SyncInfo(
                on_wait=[waits[-1]], on_update=list(si.on_update))
        return orig_cal(self, inst, original_block, old_bb_map, bb_to_exit_bb)

    tile.TileContext._commit_and_lower = patched_commit_and_lower
    bass_utils.upload_artifacts = lambda tmpdir: str(tmpdir)

    so_path = "/opt/axon/libaxon_pjrt.so"
    hook = None
    try:
        lib = ctypes.CDLL(so_path)
        if hasattr(lib, "axon_start_nrt_profile"):
            lib.axon_start_nrt_profile.argtypes = [
                ctypes.POINTER(ctypes.c_int64), ctypes.c_size_t]
            lib.axon_start_nrt_profile.restype = ctypes.c_int64
            lib.axon_stop_nrt_profile.argtypes = [ctypes.c_char_p]
            lib.axon_stop_nrt_profile.restype = ctypes.c_int64

            @contextlib.contextmanager
            def _hook(output_dir, device_ids):
                import jax
                jax.devices()
                if device_ids:
                    ids = (ctypes.c_int64 * len(device_ids))(*device_ids)
                    rc = lib.axon_start_nrt_profile(ids, len(device_ids))
                else:
                    rc = lib.axon_start_nrt_profile(None, 0)
                if rc != 0:
                    raise RuntimeError(f"axon_start_nrt_profile rc={rc}")
                try:
                    yield
                finally:
                    n = lib.axon_stop_nrt_profile(str(output_dir).encode())
                    print(f"ntff profile: {n} file(s) -> {output_dir}")

            hook = _hook
    except OSError:
        pass

    import antenv
    mod = types.ModuleType("antenv.axon_hooks")
    mod.get_axon_ntff_profile_hook = lambda: hook
    mod.set_axon_ntff_profile_hook = lambda h: None
    sys.modules["antenv.axon_hooks"] = mod
    antenv.axon_hooks = mod


SWAP_MASK = [i ^ 1 for i in range(32)]   # pair swap 2i <-> 2i+1 per quadrant


def build_nc(seq=S):
    """One SPMD program; per-core differences are input data only."""
    QC = 512                      # query-chunk width (= one PSUM bank of f32)
    n_qc = seq // QC              # query chunks
    n_kt = seq // 128             # key tiles
    n_st = seq // 128             # s tiles (o_proj M)
    KT = 8                        # contraction tiles for projections (D/128)

    nc = bass.Bass(target_bir_lowering=False)

    xT_d = nc.dram_tensor("xT", [D, seq], BF16, kind="ExternalInput")
    wq_d = nc.dram_tensor("wq", [D, DH], BF16, kind="ExternalInput")
    wk_d = nc.dram_tensor("wk", [D, DH], BF16, kind="ExternalInput")
    wv_d = nc.dram_tensor("wv", [D, DH], BF16, kind="ExternalInput")
    wo_d = nc.dram_tensor("wo", [DH, D], BF16, kind="ExternalInput")
    cosb_d = nc.dram_tensor("cosb", [128, seq], BF16, kind="ExternalInput")
    sinb_d = nc.dram_tensor("sinb", [128, seq], BF16, kind="ExternalInput")
    wm_d = nc.dram_tensor("wmask", [128, 128], BF16, kind="ExternalInput")
    out_d = nc.dram_tensor("out", [seq, D], BF16, kind="ExternalOutput")

    with tile.TileContext(nc) as tc:
        with contextlib.ExitStack() as ctx:
            res = ctx.enter_context(tc.tile_pool(name="res", bufs=1))
            ropet = ctx.enter_context(tc.tile_pool(name="ropet", bufs=3))
            pts = ctx.enter_context(tc.tile_pool(name="pts", bufs=3))
            nrm = ctx.enter_context(tc.tile_pool(name="nrm", bufs=2))
            psum = ctx.enter_context(
                tc.tile_pool(name="psum", bufs=4, space="PSUM"))

            # ---- load (host pre-casts everything to bf16) -------------------
            # DMA issue costs ~630ns per dma_start on the issuing engine, so
            # spread issue across engine queues and split xT so the first
            # projection only waits for the first half of x.
            def load_bf16(dram, cols, n_tiles, name, eng, chunks=1):
                tiles = []
                for k in range(n_tiles):
                    bt = res.tile([128, cols], BF16, name=f"{name}{k}",
                                  tag=f"{name}{k}")
                    tiles.append(bt)
                for ch in range(chunks):
                    cw = cols // chunks
                    for k in range(n_tiles):
                        eng.dma_start(
                            tiles[k][:, ch * cw:(ch + 1) * cw],
                            dram[k * 128:(k + 1) * 128, ch * cw:(ch + 1) * cw])
                return tiles

            wq = load_bf16(wq_d, DH, 8, "wq", nc.scalar)
            xT = load_bf16(xT_d, seq, 8, "xT", nc.sync, chunks=2)
            cosb = res.tile([128, seq], BF16, name="cosb", tag="cosb")
            nc.gpsimd.dma_start(cosb[:], cosb_d[:])
            sinb = res.tile([128, seq], BF16, name="sinb", tag="sinb")
            nc.gpsimd.dma_start(sinb[:], sinb_d[:])
            wm = res.tile([128, 128], BF16, name="wm", tag="wm")
            nc.gpsimd.dma_start(wm[:], wm_d[:])
            wk = load_bf16(wk_d, DH, 8, "wk", nc.sync)
            wv = load_bf16(wv_d, DH, 8, "wv", nc.sync)
            wo = load_bf16(wo_d, D, 4, "wo", nc.gpsimd)

            # ---- persistent result tiles -----------------------------------
            # qT/kT tile t: rows 0-63 = head 2t dims, rows 64-127 = head 2t+1
            # (natural interleaved even/odd order within each head).
            qT = [res.tile([128, seq], BF16, name=f"qT{m}", tag=f"qT{m}")
                  for m in range(4)]
            kTt = [res.tile([128, seq], BF16, name=f"kT{m}", tag=f"kT{m}")
                   for m in range(4)]
            v_sb = [res.tile([128, HL, DK + 1], BF16, name=f"v{t}",
                             tag=f"v{t}") for t in range(n_st)]
            aoT = [res.tile([128, seq], BF16, name=f"aoT{t}", tag=f"aoT{t}")
                   for t in range(4)]
            SCALE = 1.0 / np.sqrt(np.float32(DK))

            rs = res.tile([32, QC], F32, name="rs", tag="rs")
            nc.vector.memset(rs[:], 0.0)
            rs2 = res.tile([1, QC], F32, name="rs2", tag="rs2")

            # ---- emission units --------------------------------------------
            def proj_qk_unit(w_tiles, dst, t, qc, evac_on_act=True):
                # 8 MMs for one 128-row tile (2 heads), then rope:
                # psum->bf16 evacuate (ACT when ACT is idle, DVE during the
                # exp-bound attention phases), DVE pair-swap shuffle
                # + 2 mul + add at bf16 2x rate.
                pst = psum.tile([128, QC], F32, tag="ps", name="pj")
                for k in range(KT):
                    nc.tensor.matmul(
                        pst[:],
                        w_tiles[k][:, t * 128:(t + 1) * 128],
                        xT[k][:, qc * QC:(qc + 1) * QC],
                        start=(k == 0), stop=(k == KT - 1))
                t0 = ropet.tile([128, QC], BF16, tag="rope0", name="t0")
                if evac_on_act:
                    nc.scalar.copy(t0[:], pst[:])
                else:
                    nc.vector.tensor_copy(t0[:], pst[:])
                t1 = ropet.tile([128, QC], BF16, tag="rope1", name="t1")
                nc.vector.stream_shuffle(t1[:], t0[:], SWAP_MASK)
                cs = cosb[:, qc * QC:(qc + 1) * QC]
                sn = sinb[:, qc * QC:(qc + 1) * QC]
                ta = ropet.tile([128, QC], BF16, tag="rope2", name="ta")
                nc.vector.tensor_mul(ta[:], t0[:], cs)
                tb = ropet.tile([128, QC], BF16, tag="rope3", name="tb")
                nc.vector.tensor_mul(tb[:], t1[:], sn)
                nc.vector.tensor_add(
                    dst[t][:, qc * QC:(qc + 1) * QC], ta[:], tb[:])

            def v_unit(st_i):
                # 8 MMs: v projection for one s-tile + strided copy + ones.
                pst = psum.tile([128, DH], F32, tag="ps", name="pv_proj")
                for k in range(KT):
                    nc.tensor.matmul(
                        pst[:],
                        xT[k][:, st_i * 128:(st_i + 1) * 128],
                        wv[k][:],
                        start=(k == 0), stop=(k == KT - 1))
                vt = v_sb[st_i]
                nc.vector.tensor_copy(
                    vt[:, :, 0:DK],
                    pst[:].rearrange("p (h d) -> p h d", h=HL))
                nc.vector.memset(vt[:, :, DK:DK + 1], 1.0)

            def oproj_unit(st_i, oc):
                # 4 MMs: one o_proj output tile.
                pso = psum.tile([128, 512], F32, tag="ps", name="pso")
                for k4 in range(4):
                    nc.tensor.matmul(
                        pso[:],
                        aoT[k4][:, st_i * 128:(st_i + 1) * 128],
                        wo[k4][:, oc * 512:(oc + 1) * 512],
                        start=(k4 == 0), stop=(k4 == 3))
                ot = pts.tile([128, 512], BF16, tag="ot", name="ot", bufs=3)
                nc.vector.tensor_copy(ot[:], pso[:])
                nc.sync.dma_start(
                    out_d[st_i * 128:(st_i + 1) * 128,
                          oc * 512:(oc + 1) * 512],
                    ot[:])

            # fill queue: (mm_cost, closure). Pumped between attention tiles.
            fills = []
            fill_pos = [0]
            mm_credit = [0.0]

            def pump(n_mms):
                mm_credit[0] += n_mms
                while (fill_pos[0] < len(fills)
                       and mm_credit[0] >= fills[fill_pos[0]][0]):
                    cost, fn = fills[fill_pos[0]]
                    fn()
                    mm_credit[0] -= cost
                    fill_pos[0] += 1

            def flush_fills(upto=None):
                end = len(fills) if upto is None else upto
                while fill_pos[0] < end:
                    fills[fill_pos[0]][1]()
                    fill_pos[0] += 1
                mm_credit[0] = 0.0

            # ---- attention building blocks ---------------------------------
            def act_reciprocal(out, in_):
                # 1/d = exp(-ln d): Ln and Exp share one ACT table set, so no
                # ~2.7us table reload per call; DVE RECIPROCAL measures 3.3us
                # per [1,512] (iterative divide) so ACT is the fast path.
                nc.scalar.activation(
                    rs2[0:1, :], in_, mybir.ActivationFunctionType.Ln)
                nc.scalar.activation(
                    out, rs2[0:1, :], mybir.ActivationFunctionType.Exp,
                    scale=-1.0)

            def emit_norm(pv_pair, t, qc):
                for i in range(2):
                    h = 2 * t + i
                    act_reciprocal(rs[0:1, :], pv_pair[i][DK:DK + 1, :])
                    rbc = nrm.tile([64, QC], F32, tag="rbc", name="rbc")
                    nc.vector.stream_shuffle(rbc[0:32, :], rs[:, :], [0] * 32)
                    nc.vector.stream_shuffle(rbc[32:64, :], rs[:, :], [0] * 32)
                    nc.vector.tensor_mul(
                        aoT[t][i * 64:i * 64 + 64, qc * QC:(qc + 1) * QC],
                        pv_pair[i][0:DK, :], rbc[:])

            def att_block(qc, t, pump_rate=2):
                qt, kt_t = qT[t], kTt[t]
                pv_ps = [psum.tile([DK + 1, QC], F32, tag="ps", name="pv_ps")
                         for _ in range(2)]
                kt_hi = min(n_kt, 4 * (qc + 1))

                def emit_sc(kt):
                    r = kt - 4 * qc
                    c0 = 128 * r if r > 0 else 0
                    sc2 = psum.tile([128, 2, QC], F32, tag="sc2", name="sc2",
                                    bufs=2)
                    for i in range(2):
                        rb = 64 * i
                        nc.tensor.matmul(
                            sc2[:, i, c0:QC],
                            kt_t[rb:rb + 64, kt * 128:(kt + 1) * 128],
                            qt[rb:rb + 64, qc * QC + c0:(qc + 1) * QC],
                            start=True, stop=True, tile_position=(rb, 0))
                    return kt, c0, sc2

                def emit_px(kt, c0, sc2):
                    r = kt - 4 * qc
                    pt2 = pts.tile([128, 2, QC], BF16, tag="pts", name="pt2")
                    nc.scalar.activation(
                        pt2[:, :, c0:QC], sc2[:, :, c0:QC],
                        mybir.ActivationFunctionType.Exp, scale=SCALE)
                    if r >= 0:
                        for i in range(2):
                            nc.vector.tensor_mul(
                                pt2[:, i, c0:c0 + 128], pt2[:, i, c0:c0 + 128],
                                wm[:, :])
                    for i in range(2):
                        h = 2 * t + i
                        nc.tensor.matmul(
                            pv_ps[i][:, c0:QC],
                            v_sb[kt][:, h, :],
                            pt2[:, i, c0:QC],
                            start=(kt == 0), stop=(kt == kt_hi - 1))

                prev = None
                for kt in range(kt_hi):
                    cur = emit_sc(kt)
                    if prev is not None:
                        emit_px(*prev)
                        pump(pump_rate)
                    prev = cur
                emit_px(*prev)
                pump(pump_rate)
                emit_norm(pv_ps, t, qc)

            # ---- schedule ---------------------------------------------------
            # pre-phase: tiles 0-1 q/k projections + first v tiles (dense)
            for qc in range(n_qc):
                proj_qk_unit(wq, qT, 0, qc)
                proj_qk_unit(wq, qT, 1, qc)
            for qc in range(n_qc):
                proj_qk_unit(wk, kTt, 0, qc)
                proj_qk_unit(wk, kTt, 1, qc)
            for t in range(min(4, n_st)):
                v_unit(t)

            # fill queue for phase A: remaining v tiles, tile 2-3 projections
            for t in range(4, n_st):
                fills.append((8, lambda t=t: v_unit(t)))
            v_fill_end = len(fills)
            for qc in range(n_qc):
                fills.append(
                    (8, lambda qc=qc: proj_qk_unit(wq, qT, 2, qc, False)))
                fills.append(
                    (8, lambda qc=qc: proj_qk_unit(wq, qT, 3, qc, False)))
            for qc in range(n_qc):
                fills.append(
                    (8, lambda qc=qc: proj_qk_unit(wk, kTt, 2, qc, False)))
                fills.append(
                    (8, lambda qc=qc: proj_qk_unit(wk, kTt, 3, qc, False)))
            qk1_fill_end = len(fills)

            def ensure_v(qc):
                # v tiles up to 4*(qc+1) must exist before attention reads them
                need = min(4 * (qc + 1), n_st) - 4
                if need > 0:
                    flush_fills(upto=min(need, v_fill_end))

            # phase A: tiles 0-1 attention (+ tiles 2-3 qc0/qc1 at the end)
            for qc in range(n_qc):
                ensure_v(qc)
                pr = 3 if qc >= 2 else 2
                att_block(qc, 0, pump_rate=pr)
                att_block(qc, 1, pump_rate=pr)
            flush_fills(upto=qk1_fill_end)   # tile 2-3 q/k must be ready now
            for qc in range(min(2, n_qc)):
                ensure_v(qc)
                att_block(qc, 2)
                att_block(qc, 3)
                for st_i in range(4 * qc, min(4 * (qc + 1), n_st)):
                    for oc in range(2):
                        fills.append(
                            (4, lambda s=st_i, o=oc: oproj_unit(s, o)))

            # phase B: heavy tile 2-3 chunks with o_proj as PE filler.
            # qc=3 runs before qc=2 so the final qc's o_proj DMAs overlap
            # qc=2's attention instead of trailing the whole kernel.
            for qc in [3, 2][:max(0, n_qc - 2)]:
                ensure_v(qc)
                att_block(qc, 2)
                att_block(qc, 3)
                for st_i in range(4 * qc, min(4 * (qc + 1), n_st)):
                    for oc in range(2):
                        fills.append(
                            (4, lambda s=st_i, o=oc: oproj_unit(s, o)))
            flush_fills()
    return nc


def prepare_inputs(x, q_proj, k_proj, v_proj, o_proj, token_positions, seq=S):
    """Shard + lay out host-side. Returns one in_map per core."""
    from ml_dtypes import bfloat16
    x = np.asarray(x, dtype=np.float32)
    q_proj = np.asarray(q_proj, dtype=np.float32)
    k_proj = np.asarray(k_proj, dtype=np.float32)
    v_proj = np.asarray(v_proj, dtype=np.float32)
    o_proj = np.asarray(o_proj, dtype=np.float32)
    pos = np.asarray(token_positions)

    # rope tables (exactly mirrors reference._rope_tables + gather)
    dims = np.arange(0, DK, 2, dtype=np.float32)
    freqs = 1.0 / THETA ** (dims / DK)
    t = np.arange(2048, dtype=np.float32)
    angles = np.outer(t, freqs)                      # (2048, 32)
    cos_tab = np.cos(angles)[pos].astype(np.float32)  # (seq, 32)
    sin_tab = np.sin(angles)[pos].astype(np.float32)
    # interleaved layout: row r uses frequency (r % 64) // 2; sin sign is
    # -1 on even rows (out_e = e*cos - o*sin) and +1 on odd rows.
    cos_il = np.repeat(np.ascontiguousarray(cos_tab.T), 2, axis=0)  # (64, seq)
    sin_il = np.repeat(np.ascontiguousarray(sin_tab.T), 2, axis=0)
    sign = np.where(np.arange(64) % 2 == 0, -1.0, 1.0)[:, None].astype(
        np.float32)
    sin_il = sin_il * sign
    cosb = np.tile(cos_il, (2, 1))                   # (128, seq)
    sinb = np.tile(sin_il, (2, 1))

    # shifted causal mask for the 128-wide diagonal block: wm[k, c] = c >= k
    kk = np.arange(128)[:, None]
    cc = np.arange(128)[None, :]
    wm = (cc >= kk).astype(np.float32)

    in_maps = []
    for c in range(N_CORES):
        b, hg = c // 2, c % 2
        hslice = slice(hg * DH, (hg + 1) * DH)
        in_maps.append({
            "xT": np.ascontiguousarray(x[b, :seq, :].T).astype(bfloat16),
            "wq": np.ascontiguousarray(q_proj[:, hslice]).astype(bfloat16),
            "wk": np.ascontiguousarray(k_proj[:, hslice]).astype(bfloat16),
            "wv": np.ascontiguousarray(v_proj[:, hslice]).astype(bfloat16),
            "wo": np.ascontiguousarray(o_proj[hslice, :]).astype(bfloat16),
            "cosb": cosb[:, :seq].astype(bfloat16),
            "sinb": sinb[:, :seq].astype(bfloat16),
            "wmask": wm.astype(bfloat16),
        })
    return in_maps


def run(inputs, seq=S, trace=False, tmpdir=None):
    _install_patches()
    if seq not in _NC_CACHE:
        _NC_CACHE[seq] = build_nc(seq)
    nc = _NC_CACHE[seq]
    in_maps = prepare_inputs(**inputs, seq=seq)
    kw = {}
    if trace:
        kw = dict(trace=True, tmpdir=tmpdir)
    res = bass_utils.run_bass_kernel_spmd(
        nc, in_maps, core_ids=list(range(N_CORES)), **kw)
    parts = [np.asarray(res.results[c]["out"], dtype=np.float32)
             for c in range(N_CORES)]
    out = np.stack([parts[2 * b] + parts[2 * b + 1] for b in range(B)])
    return out, res


def kernel(x, q_proj, k_proj, v_proj, o_proj, token_positions):
    out, _ = run(dict(x=x, q_proj=q_proj, k_proj=k_proj, v_proj=v_proj,
                      o_proj=o_proj, token_positions=token_positions))
    return out
